# revision 4
# baseline (speedup 1.0000x reference)
"""Overlapping-windows (conv1d-identity unfold) kernel for Trainium2.

out[b*T + t, w*C + c] = x[b, t + w - CTX, c]  (zero-padded in t), i.e. each
output row is a contiguous 494-element window of the zero-padded, flattened
per-batch [T + 2*CTX, C] array starting at t*C.

Strategy (pure DMA, no compute):
  - Shard batch dim across 8 cores (8 batches/core).
  - Per core, stage the padded input in SBUF as 128 partitions =
    8 batches x 16 time-chunks; partition (b, j) holds padded rows
    [j*K, j*K + K + 2*CTX) of batch b  (K = 125 rows, 3718 f32 / partition).
  - One SBUF->DRAM DMA writes the whole per-core output [16000, 494]:
    source AP reads overlapping windows (row stride C=26, length 494) within
    each partition; destination is fully contiguous.
"""

import numpy as np

N_CTX = 9
C = 26
W = 2 * N_CTX + 1          # 19
ROWLEN = W * C             # 494
B, T = 64, 2000
N_CORES = 8
B_C = B // N_CORES         # 8 batches per core
NCHUNK = 16                # time-chunks per batch -> 8*16 = 128 partitions
K = T // NCHUNK            # 125 rows per chunk
PF = (K + 2 * N_CTX) * C   # 3718 f32 per partition (chunk + halo)
EDGE = (K + N_CTX) * C     # 3484 f32 (edge chunks have one-sided halo)


def _build_nc():
    import concourse.bass as bass
    import concourse.mybir as mybir

    nc = bass.Bass(target_bir_lowering=False)
    x = nc.dram_tensor("x", [B_C, T, C], mybir.dt.float32, kind="ExternalInput")
    out = nc.dram_tensor(
        "out", [B_C * T, ROWLEN], mybir.dt.float32, kind="ExternalOutput"
    )

    with (
        nc.sbuf_tensor("xs", [128, PF], mybir.dt.float32) as xs,
        nc.semaphore("dma_sem") as dma_sem,
        nc.semaphore("msem") as msem,
        nc.Block() as block,
    ):

        @block.vector
        def _(vector):
            # Zero the halo strips that fall outside [0, T): head of chunk 0
            # and tail of chunk 15 for every batch. Full-tile memset is
            # simplest and still cheap (~4us).
            vector.memset(bass.AP(xs, 0, [[PF, 128], [1, PF]]), 0.0).then_inc(
                msem, 1
            )

        @block.sync
        def _(sync):
            sync.wait_ge(msem, 1)
            # chunk j=0 per batch: rows [0, K+CTX) land at offset CTX*C
            sync.dma_start(
                bass.AP(xs, N_CTX * C, [[NCHUNK * PF, B_C], [1, EDGE]]),
                bass.AP(x, 0, [[T * C, B_C], [1, EDGE]]),
            ).then_inc(dma_sem, 16)
            # chunks j=1..14 per batch: rows [j*K-CTX, j*K-CTX+143), full
            # lines. One DMA per batch — SBUF APs advance partitions only on
            # the first dim, so a second partition-crossing dim is illegal.
            for b in range(B_C):
                sync.dma_start(
                    bass.AP(
                        xs,
                        (b * NCHUNK + 1) * PF,
                        [[PF, NCHUNK - 2], [1, PF]],
                    ),
                    bass.AP(
                        x,
                        (b * T + K - N_CTX) * C,
                        [[K * C, NCHUNK - 2], [1, PF]],
                    ),
                ).then_inc(dma_sem, 16)
            # chunk j=15 per batch: rows [15*K-CTX, T) land at offset 0
            sync.dma_start(
                bass.AP(xs, (NCHUNK - 1) * PF, [[NCHUNK * PF, B_C], [1, EDGE]]),
                bass.AP(
                    x,
                    ((NCHUNK - 1) * K - N_CTX) * C,
                    [[T * C, B_C], [1, EDGE]],
                ),
            ).then_inc(dma_sem, 16)

            sync.wait_ge(dma_sem, 16 * (2 + B_C))
            # The whole output in one DMA: partition p covers output rows
            # [p*K, (p+1)*K); row r reads xs[p, r*C : r*C + ROWLEN]
            # (overlapping strided reads), dst is contiguous.
            sync.dma_start(
                bass.AP(out, 0, [[K * ROWLEN, 128], [ROWLEN, K], [1, ROWLEN]]),
                bass.AP(xs, 0, [[PF, 128], [C, K], [1, ROWLEN]]),
            ).then_inc(dma_sem, 16)
            sync.wait_ge(dma_sem, 16 * (3 + B_C))

    return nc


def kernel(x: np.ndarray) -> np.ndarray:
    from concourse.bass_utils import run_bass_kernel_spmd

    x = np.ascontiguousarray(np.asarray(x), dtype=np.float32)
    assert x.shape == (B, T, C), x.shape

    nc = _build_nc()
    in_maps = [{"x": x[i * B_C : (i + 1) * B_C]} for i in range(N_CORES)]
    res = run_bass_kernel_spmd(nc, in_maps, core_ids=list(range(N_CORES)))
    return np.concatenate([r["out"] for r in res.results], axis=0)


# revision 5
# speedup vs baseline: 1.1878x; 1.1878x over previous
"""Overlapping-windows (conv1d-identity unfold) kernel for Trainium2.

out[b*T + t, w*C + c] = x[b, t + w - CTX, c]  (zero-padded in t), i.e. each
output row is a contiguous 494-element window of the zero-padded, flattened
per-batch [T + 2*CTX, C] array starting at t*C.

Strategy (pure DMA, no compute):
  - Shard batch dim across 8 cores (8 batches/core).
  - Per core, stage the padded input in SBUF as 128 partitions =
    8 batches x 16 time-chunks; partition (b, j) holds padded rows
    [j*K, j*K + K + 2*CTX) of batch b  (K = 125 rows, 3718 f32 / partition).
  - One SBUF->DRAM DMA writes the whole per-core output [16000, 494]:
    source AP reads overlapping windows (row stride C=26, length 494) within
    each partition; destination is fully contiguous.
"""

import numpy as np

N_CTX = 9
C = 26
W = 2 * N_CTX + 1          # 19
ROWLEN = W * C             # 494
B, T = 64, 2000
N_CORES = 8
B_C = B // N_CORES         # 8 batches per core
NCHUNK = 16                # time-chunks per batch -> 8*16 = 128 partitions
K = T // NCHUNK            # 125 rows per chunk
PF = (K + 2 * N_CTX) * C   # 3718 f32 per partition (chunk + halo)
EDGE = (K + N_CTX) * C     # 3484 f32 (edge chunks have one-sided halo)


def _build_nc():
    import concourse.bass as bass
    import concourse.mybir as mybir

    nc = bass.Bass(target_bir_lowering=False)
    x = nc.dram_tensor("x", [B_C, T, C], mybir.dt.float32, kind="ExternalInput")
    out = nc.dram_tensor(
        "out", [B_C * T, ROWLEN], mybir.dt.float32, kind="ExternalOutput"
    )

    with (
        nc.sbuf_tensor("xs", [128, PF], mybir.dt.float32) as xs,
        nc.semaphore("dma_sem") as dma_sem,
        nc.semaphore("msem") as msem,
        nc.Block() as block,
    ):

        @block.vector
        def _(vector):
            # Zero the halo strips that fall outside [0, T): head of chunk 0
            # and tail of chunk 15 for every batch. Full-tile memset is
            # simplest and still cheap (~4us).
            vector.memset(bass.AP(xs, 0, [[PF, 128], [1, PF]]), 0.0).then_inc(
                msem, 1
            )

        @block.sync
        def _(sync):
            sync.wait_ge(msem, 1)
            # chunk j=0 per batch: rows [0, K+CTX) land at offset CTX*C
            sync.dma_start(
                bass.AP(xs, N_CTX * C, [[NCHUNK * PF, B_C], [1, EDGE]]),
                bass.AP(x, 0, [[T * C, B_C], [1, EDGE]]),
            ).then_inc(dma_sem, 16)
            # chunks j=1..14 per batch: rows [j*K-CTX, j*K-CTX+143), full
            # lines. One DMA per batch — SBUF APs advance partitions only on
            # the first dim, so a second partition-crossing dim is illegal.
            for b in range(B_C):
                sync.dma_start(
                    bass.AP(
                        xs,
                        (b * NCHUNK + 1) * PF,
                        [[PF, NCHUNK - 2], [1, PF]],
                    ),
                    bass.AP(
                        x,
                        (b * T + K - N_CTX) * C,
                        [[K * C, NCHUNK - 2], [1, PF]],
                    ),
                ).then_inc(dma_sem, 16)
            # chunk j=15 per batch: rows [15*K-CTX, T) land at offset 0
            sync.dma_start(
                bass.AP(xs, (NCHUNK - 1) * PF, [[NCHUNK * PF, B_C], [1, EDGE]]),
                bass.AP(
                    x,
                    ((NCHUNK - 1) * K - N_CTX) * C,
                    [[T * C, B_C], [1, EDGE]],
                ),
            ).then_inc(dma_sem, 16)

            sync.wait_ge(dma_sem, 16 * (2 + B_C))
            # Output: partition p covers rows [p*K, (p+1)*K); row r reads
            # xs[p, r*C : r*C + ROWLEN] (overlapping strided reads), dst is
            # contiguous. Split across both HWDGE rings (sync + scalar) so
            # two descriptor generators run in parallel.
            r0, r1 = K // 2, K - K // 2
            sync.dma_start(
                bass.AP(out, 0, [[K * ROWLEN, 128], [ROWLEN, r0], [1, ROWLEN]]),
                bass.AP(xs, 0, [[PF, 128], [C, r0], [1, ROWLEN]]),
            ).then_inc(dma_sem, 16)
            sync.wait_ge(dma_sem, 16 * (4 + B_C))

        @block.scalar
        def _(scalar):
            scalar.wait_ge(dma_sem, 16 * (2 + B_C))
            r0, r1 = K // 2, K - K // 2
            scalar.dma_start(
                bass.AP(
                    out,
                    r0 * ROWLEN,
                    [[K * ROWLEN, 128], [ROWLEN, r1], [1, ROWLEN]],
                ),
                bass.AP(xs, r0 * C, [[PF, 128], [C, r1], [1, ROWLEN]]),
            ).then_inc(dma_sem, 16)
            scalar.wait_ge(dma_sem, 16 * (4 + B_C))

    return nc


def kernel(x: np.ndarray) -> np.ndarray:
    from concourse.bass_utils import run_bass_kernel_spmd

    x = np.ascontiguousarray(np.asarray(x), dtype=np.float32)
    assert x.shape == (B, T, C), x.shape

    nc = _build_nc()
    in_maps = [{"x": x[i * B_C : (i + 1) * B_C]} for i in range(N_CORES)]
    res = run_bass_kernel_spmd(nc, in_maps, core_ids=list(range(N_CORES)))
    return np.concatenate([r["out"] for r in res.results], axis=0)
